# revision 12
# baseline (speedup 1.0000x reference)
"""Trainium2 Bass kernel: 3x3 conv (stride 1, pad 1) via shifted-matmul, bf16.

Full problem: x (32, 18, 256, 256) f32, weight (64, 18, 3, 3), bias (64,)
-> out (32, 64, 256, 256).  Data-parallel over batch: 8 cores x 4 images.

Per-core algorithm (v3, K=18 single-copy loads + 8-way TensorE packing):
  - All matmul data is bf16 (tolerance 2e-2; bf16 lands ~3e-3): halves
    HBM bytes vs f32 and unlocks array tiling (fp32r pins PSUM base 0).
  - Four row-lanes, one per image: lane j holds its strip of x on SBUF
    partitions [32j, 32j+18) as G [18, R+2, 258] (partition = channel,
    NO kh replication -> HBM loads are 1x, 9.8 MB/core instead of 28.6).
  - Each output row pair accumulates 9 matmuls (kh x kw taps): kh is a
    rhs ROW offset, kw a COLUMN offset into the same G tile; K=18.
    K<=32, M=64 -> tile_size (32, 64): 8 independent PE quadrant-tiles
    (4 row groups x 2 column groups).  Consecutive matmuls rotate over
    all 8 tiles so 8 stream concurrently (~27ns/MM effective).
  - tile_position passed explicitly: (32j, 64ch); LDW-opt must stay off
    (walrus rejects column-tile LDWEIGHTS), so every matmul carries its
    own cheap [18, 64] LDWEIGHTS that the PE overlaps across quadrants.
  - PSUM: 8 banks [128, 512] per round; bank = lane j's row pairs
    (4q+2b | 32+4q+2b) in the lo | hi partition half.  8 rounds/strip.
  - Drain: one ACT-or-DVE op per bank (f32 PSUM -> bf16 + bias).
  - Stores: per lane-strip 2 DMAs (one per partition half, outer dim =
    64 channels -> all 16 SDMA engines), 16 KB/partition contiguous
    runs, alternating scalar/gpsimd rings; loads ride the sync ring.
"""

import re
import numpy as np

import bass_rust
import concourse.bass as bass
import concourse.mybir as mybir
from concourse.tile import TileContext


# ---------------------------------------------------------------------------
# TileContext drain patch: this walrus build rejects an InstDrain carrying
# more than ~2 sync waits ("Too many sync wait commands").  Re-emit the
# end-of-kernel global-clock waits as one nop per semaphore, then drain.
# ---------------------------------------------------------------------------
def _patched_drain_and_barrier(self, tick_clock, wait_clock):
    gc = tick_clock.global_clock
    vals = [int(s) for s in re.findall(r"\d+", repr(gc))]
    for i, v in enumerate(vals):
        if v > 0:
            c = bass_rust.VectorClock()
            c.require_at_least(i, v)
            nop = self.nc.sync.nop(nofuse=True, hint=f"drain_wait_{i}")
            wait_clock.add_sem_waits(nop.ins, bass_rust.ScopedClock({None: c}))
    self.nc.sync.drain()

    self.nc.all_engine_barrier()
    assert self.sems is not None
    popped = self.nc._tile_sem_poison_stack.pop()
    assert popped is self._sem_poison
    self.nc.clear_and_free_semaphores(list(self.sems.allocated().values()))
    self.nc.all_engine_barrier()


TileContext._drain_and_barrier = _patched_drain_and_barrier


def _patch_ldw_opt(enable=True):
    """Toggle walrus's load-weights elision (the pipeline passes
    --enable-ldw-opt=false).  NOTE: ldw-opt rejects LDWEIGHTS with a
    nonzero column tile position, so it must stay off for kernels using
    PSUM-half column packing (tile_position[1] == 64)."""
    import concourse.bass_utils as _bu

    if getattr(_bu, "_ldw_opt_patched", None) == enable:
        return
    _orig = getattr(_bu, "_ldw_orig_run_command", _bu.run_command)
    _bu._ldw_orig_run_command = _orig

    def _patched(cmd, *a, **kw):
        if enable:
            cmd = [
                "--enable-ldw-opt=true" if c == "--enable-ldw-opt=false" else c
                for c in cmd
            ]
        return _orig(cmd, *a, **kw)

    _bu.run_command = _patched
    _bu._ldw_opt_patched = enable


def _split_excess_waits(nc, max_waits=1):
    """This walrus build allows very few sync waits per instruction.
    Hoist excess waits onto same-engine nops placed just before."""
    for f in nc.m.functions:
        for bb in f.blocks:
            out = []
            changed = False
            for inst in bb.instructions:
                si = inst.sync_info
                waits = list(si.on_wait) if si and si.on_wait else []
                if len(waits) > max_waits:
                    changed = True
                    extras, keep = waits[:-max_waits], waits[-max_waits:]
                    for j, w in enumerate(extras):
                        nop = mybir.InstNoOp(
                            name=f"{inst.name}_xw{j}", ins=[], outs=[]
                        )
                        nop.engine = inst.engine
                        nop.sync_info = mybir.SyncInfo(on_wait=[w], on_update=[])
                        out.append(nop)
                    inst.sync_info = mybir.SyncInfo(
                        on_wait=keep,
                        on_update=list(si.on_update) if si.on_update else [],
                    )
                out.append(inst)
            if changed:
                bb.instructions = out


# ---------------------------------------------------------------------------
# Kernel builder
# ---------------------------------------------------------------------------
F32 = mybir.dt.float32
BF16 = mybir.dt.bfloat16


def build_conv_nc(
    n_img=4,
    H=256,
    W=256,
    R=64,
    C_IN=18,
    C_OUT=64,
    act_frac=4,  # of 8 drains per round, how many go to ACT (rest DVE)
    store_rings=("scalar", "gpsimd", "scalar", "gpsimd"),
    gbufs=2,
    obufs=8,
):
    """Build the per-core Bass program. Returns nc."""
    assert n_img == 4 and H % R == 0 and R % 8 == 0
    Wp = W + 2
    Hp = H + 2
    Rp = R + 2

    nc = bass.Bass()
    # x is host-pre-padded to [Hp, Wp] (zero border): every strip load is
    # one fully-contiguous [Rp, Wp] block per channel.
    x = nc.dram_tensor("x", [n_img, C_IN, Hp, Wp], BF16, kind="ExternalInput")
    wT = nc.dram_tensor("wT", [96 + C_IN, 9, C_OUT], BF16, kind="ExternalInput")
    bias2 = nc.dram_tensor("bias2", [2 * C_OUT, 1], F32, kind="ExternalInput")
    y = nc.dram_tensor("y", [n_img, C_OUT, H, W], BF16, kind="ExternalOutput")

    n_strips = H // R
    rounds = R // 8  # 8 output rows per lane per round
    x_ap = x[:]
    y_ap = y[:]
    ring = {"scalar": nc.scalar, "sync": nc.sync, "gpsimd": nc.gpsimd}

    with TileContext(nc) as tc:
        with (
            tc.tile_pool(name="wpool", bufs=1) as wpool,
            tc.tile_pool(name="gpool", bufs=gbufs) as gpool,
            tc.tile_pool(name="opool", bufs=obufs) as opool,
            tc.tile_pool(name="psum", bufs=8, space="PSUM") as pspool,
        ):
            wsb = wpool.tile([96 + C_IN, 9, C_OUT], BF16, tag="wsb")
            bsb = wpool.tile([2 * C_OUT, 1], F32, tag="bsb")
            nc.sync.dma_start(out=wsb[:, :, :], in_=wT[:])
            nc.sync.dma_start(out=bsb[:], in_=bias2[:])

            tile_idx = 0
            for s in range(n_strips):
                h0 = s * R
                G_t = gpool.tile([96 + C_IN, Rp, Wp], BF16, tag="G")
                for j in range(n_img):
                    # Per lane: channels on partitions [32j, 32j+18),
                    # padded rows [h0, h0+R+2) contiguous per channel.
                    src = bass.AP(
                        tensor=x_ap.tensor,
                        offset=j * C_IN * Hp * Wp + h0 * Wp,
                        ap=[[Hp * Wp, C_IN], [1, Rp * Wp]],
                    )
                    jb = 32 * j
                    nc.sync.dma_start(out=G_t[jb : jb + C_IN], in_=src)
                OBs = [
                    opool.tile([2 * C_OUT, R // 4, 512], BF16, tag="OB", name="OB")
                    for _ in range(n_img)
                ]
                for q in range(rounds):
                    PTs = [
                        [
                            pspool.tile([2 * C_OUT, 512], F32, tag="PT", name="PT")
                            for _ in range(2)
                        ]
                        for _ in range(n_img)
                    ]
                    # 9 taps x 16 (lane, col-half, bank) MMs; consecutive
                    # MMs rotate across the 8 array quadrants (j, ch).
                    for tap in range(9):
                        kh, kwv = divmod(tap, 3)
                        for b in range(2):
                            for ch in range(2):
                                for j in range(n_img):
                                    jb = 32 * j
                                    l = (R // 2) * ch + 4 * q + 2 * b
                                    nc.tensor.matmul(
                                        PTs[j][b][64 * ch : 64 * ch + 64],
                                        wsb[jb : jb + C_IN, tap, :],
                                        G_t[
                                            jb : jb + C_IN,
                                            l + kh : l + kh + 2,
                                            kwv : kwv + W,
                                        ],
                                        start=(tap == 0),
                                        stop=(tap == 8),
                                        skip_group_check=True,
                                        tile_position=(jb, 64 * ch),
                                    )
                    # Drain: one op per bank [128, 512] (f32 PSUM -> bf16
                    # staging + bias), split across ACT and DVE.
                    for j in range(n_img):
                        for b in range(2):
                            PT = PTs[j][b]
                            dst = OBs[j][:, 2 * q + b, :]
                            if tile_idx % 8 < act_frac:
                                nc.scalar.activation(
                                    dst,
                                    PT[:],
                                    mybir.ActivationFunctionType.Identity,
                                    bias=bsb[0 : 2 * C_OUT],
                                )
                            else:
                                nc.vector.tensor_scalar_add(
                                    dst, PT[:], bsb[0 : 2 * C_OUT]
                                )
                            tile_idx += 1
                # Store: per lane, 2 DMAs (one per partition half g, outer
                # dim = 64 channels so the DMA splitter engages all 16
                # SDMA engines).  Partition (g, c) holds output rows
                # [h0 + (R/2)g, h0 + (R/2)(g+1)) as one contiguous run.
                for j in range(n_img):
                    eng = ring[store_rings[j]]
                    for g in range(2):
                        dst = bass.AP(
                            tensor=y_ap.tensor,
                            offset=j * C_OUT * H * W + (h0 + (R // 2) * g) * W,
                            ap=[[H * W, C_OUT], [1, (R // 2) * W]],
                        )
                        eng.dma_start(
                            out=dst, in_=OBs[j][64 * g : 64 * g + 64]
                        )
    return nc


# ---------------------------------------------------------------------------
# Host-side entry point
# ---------------------------------------------------------------------------
N_CORES = 8


def prep_inputs(x_shard, weight, bias):
    import ml_dtypes

    bf16 = ml_dtypes.bfloat16
    # lhsT row c = weight[:, c, kh, kw] for tap = 3*kh + kw; duplicated
    # on partitions [32j, 32j+18) for each of the 4 lanes.
    w9 = np.ascontiguousarray(
        np.transpose(weight, (1, 2, 3, 0)).reshape(18, 9, 64)
    ).astype(bf16)
    wT = np.zeros((114, 9, 64), bf16)
    for j in range(4):
        wT[32 * j : 32 * j + 18] = w9
    bias2 = np.concatenate([bias, bias]).reshape(128, 1).astype(np.float32)
    n, c, H, W = x_shard.shape
    x_pad = np.zeros((n, c, H + 2, W + 2), bf16)
    x_pad[:, :, 1 : H + 1, 1 : W + 1] = x_shard.astype(bf16)
    return {"x": x_pad, "wT": wT, "bias2": bias2}


def run(x, weight, bias, trace=False, ldw_opt=False, **build_kwargs):
    from concourse.bass_utils import run_bass_kernel_spmd

    x = np.asarray(x, dtype=np.float32)
    weight = np.asarray(weight, dtype=np.float32)
    bias = np.asarray(bias, dtype=np.float32)

    B = x.shape[0]
    per = B // N_CORES
    nc = build_conv_nc(n_img=per, **build_kwargs)
    _split_excess_waits(nc)
    _patch_ldw_opt(ldw_opt)
    in_maps = [
        prep_inputs(x[i * per : (i + 1) * per], weight, bias)
        for i in range(N_CORES)
    ]
    res = run_bass_kernel_spmd(nc, in_maps, list(range(N_CORES)), trace=trace)
    y = np.concatenate(
        [np.asarray(res.results[i]["y"], dtype=np.float32) for i in range(N_CORES)],
        axis=0,
    )
    return y, res


def kernel(x, weight, bias):
    return run(x, weight, bias)[0]
